# revision 1
# baseline (speedup 1.0000x reference)
import numpy as np

N=4096; C=1024; INTER=128; R=128; RR=R*R; GC=256; NCORES=8; NB=N//NCORES
PW=130; HR=R//NCORES           # 16 output h-rows per core
WINR=HR+2                      # 18 padded rows in window
WIN=WINR*PW                    # 2340
QT=(WIN+127)//128              # 19 k-tiles for q
QPAD=QT*128                    # 2432
PWIN=2694                      # window read span
PGLOB=17280                    # padded p buffer (guard 131 + 16900 + tail)
AGS=NB*(INTER+1)+HR*R          # 512*129+2048 = 68096
ARS=9*GC+C                     # 2304+1024 = 3328
KT=C//128                      # 8

_cache = {}

def _fold(p):
    f32=np.float32
    out={}
    mcw1=p['m_cw'][:INTER]; mcw2=p['m_cw'][INTER:]
    xv=np.zeros((C,6),f32); sc=np.zeros((1,8),f32)
    xv[:,0]=p['m_tw'].T@mcw1; sc[0,0]=p['m_tb']@mcw1            # a
    for j in range(3):
        c1=p['pr_cw'][j,:INTER]; c2=p['pr_cw'][j,INTER:]
        xv[:,1+j]=p['pr_tw'][j].T@c1
        sc[0,1+j]=p['pr_tb'][j]@c1+p['pr_pb'][j]@c2
    xv[:,4]=p['ba_tw'].T@p['ba_cw'][:INTER]
    xv[:,5]=p['m_pw'].T@mcw2; sc[0,5]=p['m_pb']@mcw2            # b
    sc[0,4]=p['ba_tb']@p['ba_cw'][:INTER]+p['ba_pb']@p['ba_cw'][INTER:]
    out['xvecs']=xv; out['sconst']=sc
    vps=np.stack([p['pr_pw'][j].T@p['pr_cw'][j,INTER:] for j in range(3)],1)
    out['vps']=vps.astype(f32)                                   # [C,3]
    out['vpm']=(p['ba_pw'].T@p['ba_cw'][INTER:]/ (2*N)).astype(f32)[:,None]  # [C,1]
    out['m_gwT']=p['m_gw'].T.copy()                              # [C,128]
    out['pr_gwT']=np.stack([p['pr_gw'][j].T for j in range(3)])  # [3,C,128]
    bg=float(p['ba_g'][0])
    out['ba_gwT']=(bg*p['ba_gw'].T/(2*N)).copy()                 # [C,128]
    sg=float(p['sp_g'][0])
    # sp_gwT: [(mh*3+mw)*GC+ic, oc] with (kh,kw)=(2-mh,2-mw), scaled by sp_g
    g=np.transpose(p['sp_gw'],(2,3,1,0))[::-1,::-1]              # [kh',kw',ic,oc] reversed
    out['sp_gwT']=np.ascontiguousarray(sg*g.reshape(9*GC,INTER))
    # w_effT [2,128,9]: w_eff[ic,kh,kw]=sum_c spcw2[c]*sp_pw[c,ic,kh,kw]
    we=np.einsum('c,cikl->ikl',p['sp_cw'][INTER:],p['sp_pw'])    # [GC,3,3]
    out['w_effT']=we.reshape(2,128,9).astype(f32)
    # biases128 [128,6]: m_gb, pr_gb0..2, ba_g*ba_gb(gm bias), sp_g*sp_gb(v bias)
    b6=np.zeros((INTER,6),f32)
    b6[:,0]=p['m_gb']; b6[:,1:4]=p['pr_gb'].T; b6[:,4]=bg*p['ba_gb']; b6[:,5]=sg*p['sp_gb']
    out['bias128']=b6
    gf=np.zeros((1,4*INTER),f32)
    for j in range(3): gf[0,j*INTER:(j+1)*INTER]=p['pr_g'][j]
    gf[0,3*INTER:]=1.0
    out['gfill']=gf
    out['mgb_row']=p['m_gb'][None,:].astype(f32)                 # [1,128] K=1 bias trick
    return out

def _shard(p):
    f32=np.float32
    gpadded=np.pad(p['global_feature'][0],((0,0),(1,1),(1,1)))   # [GC,130,130]
    ins=[]
    for k in range(NCORES):
        d={}
        rs=slice(k*NB,(k+1)*NB)
        d['xT']=np.ascontiguousarray(p['origin_feature'][rs].T)
        yt=np.stack([np.ascontiguousarray(t[rs].T) for t in
                     (p['local_feature'],p['bef_l'],p['aft_l'])])
        d['yT']=yt                                               # [3,C,NB]
        d['bafT']=np.ascontiguousarray(np.concatenate(
            [p['bef'][rs],p['aft'][rs]],0).T)                    # [C,2NB]
        gw=gpadded[:,k*HR:k*HR+WINR,:]                           # [GC,18,130]
        d['gpad']=np.ascontiguousarray(gw.reshape(2,128,WINR*PW)
                    .transpose(1,0,2).reshape(128,2*WINR*PW))
        gt=np.zeros((QPAD,GC),f32)
        gt[:WIN]=gw.reshape(GC,WIN).T
        d['gpadT']=gt.reshape(QT,128,GC)
        osel=np.zeros((NCORES,1),f32); osel[k,0]=1.0
        d['osel']=osel
        ins.append(d)
    return ins

def kernel(**inputs):
    import ml_dtypes  # noqa
    if 'nc' not in _cache:
        _cache['nc']=build()
    nc=_cache['nc']
    fold=_fold(inputs); shards=_shard(inputs)
    in_maps=[]
    for k in range(NCORES):
        m=dict(shards[k]); m.update(fold)
        in_maps.append({kk:np.ascontiguousarray(v,dtype=np.float32) for kk,v in m.items()})
    from concourse.bass_utils import run_bass_kernel_spmd
    res=run_bass_kernel_spmd(nc,in_maps,list(range(NCORES)))
    out=np.empty((N,INTER),np.float32)
    for k in range(NCORES):
        out[k*NB:(k+1)*NB]=res.results[k]['out'].T
    return out


# ---- device program builder (inlined) ----
import numpy as np
import bass_rust
import concourse.bass as bass
import concourse.bacc as bacc
import concourse.mybir as mybir
import concourse.tile as tile

F32=mybir.dt.float32
AF=mybir.ActivationFunctionType
AL=mybir.AluOpType
RG=[list(range(NCORES))]

def mkap(a,offset,dims):
    b=a.copy(); b.offset=offset
    b.ap=bass_rust.VecI64Pair([list(d) for d in dims])
    return b

def build():
    nc=bacc.Bacc("TRN2",target_bir_lowering=False,debug=False,num_devices=NCORES)
    P=lambda n,s: nc.declare_dram_parameter(n,list(s),F32,isOutput=False)
    xT=P('xT',(C,NB)); yT=P('yT',(3,C,NB)); bafT=P('bafT',(C,2*NB))
    gpad=P('gpad',(128,2*WIN)); gpadT=P('gpadT',(QT,128,GC)); osel=P('osel',(NCORES,1))
    xv=P('xvecs',(C,6)); sc=P('sconst',(1,8)); vps=P('vps',(C,3)); vpm=P('vpm',(C,1))
    mgw=P('m_gwT',(C,INTER)); prgw=P('pr_gwT',(3,C,INTER)); bagw=P('ba_gwT',(C,INTER))
    spgw=P('sp_gwT',(9*GC,INTER)); weT=P('w_effT',(2,128,9)); b6=P('bias128',(INTER,6))
    gf=P('gfill',(1,4*INTER)); mgbr=P('mgb_row',(1,INTER))
    out_ext=nc.declare_dram_parameter('out',[INTER,NB],F32,isOutput=True)

    with tile.TileContext(nc) as tc:
      with (tc.tile_pool(name="pp",bufs=1) as pp,
            tc.tile_pool(name="ww",bufs=4) as ww,
            tc.tile_pool(name="dr",bufs=1,space="DRAM") as dr,
            tc.tile_pool(name="ps_or",bufs=1,space="PSUM") as ps_or,
            tc.tile_pool(name="ps_six",bufs=1,space="PSUM") as ps_six,
            tc.tile_pool(name="ps_mid",bufs=2,space="PSUM") as ps_mid,
            tc.tile_pool(name="ps_roll",bufs=2,space="PSUM") as ps_roll,
            tc.tile_pool(name="ps_sm",bufs=1,space="PSUM") as ps_sm):
        dma=nc.sync.dma_start
        ag_in=dr.tile([AGS],F32); ag_out=dr.tile([NCORES*AGS],F32,addr_space='Shared')
        ar_in=dr.tile([ARS],F32); ar_out=dr.tile([ARS],F32,addr_space='Shared')
        p_glob=dr.tile([PGLOB],F32); p_loc=dr.tile([2816],F32)
        def ld(name,shape,src_ap):
            t=pp.tile(shape,F32,tag=name); dma(t[:],src_ap); return t
        xT_s=ld('xT',[128,KT,NB],xT.ap().rearrange("(k p) n -> p k n",p=128))
        yT_s=ld('yT',[128,3,KT,NB],yT.ap().rearrange("j (k p) n -> p j k n",p=128))
        gp_s=pp.tile([128,2,WIN],F32,tag='big',name='gp_s',padded_shape=[128,2,WIN])
        dma(gp_s[:],gpad.ap().rearrange("p (h w) -> p h w",h=2))
        xv_s=ld('xv',[128,KT,6],xv.ap().rearrange("(k p) n -> p k n",p=128))
        vp_s=ld('vp',[128,KT,3],vps.ap().rearrange("(k p) n -> p k n",p=128))
        vpm_s=ld('vpm',[128,KT,1],vpm.ap().rearrange("(k p) n -> p k n",p=128))
        mgw_s=ld('mgw',[128,KT,INTER],mgw.ap().rearrange("(k p) n -> p k n",p=128))
        pr_s=ld('pr',[128,3,KT,INTER],prgw.ap().rearrange("j (k p) n -> p j k n",p=128))
        bag_s=ld('bag',[128,KT,INTER],bagw.ap().rearrange("(k p) n -> p k n",p=128))
        spg_s=ld('spg',[128,18,INTER],spgw.ap().rearrange("(k p) n -> p k n",p=128))
        we_s=ld('we',[128,2,9],weT.ap().rearrange("h p n -> p h n"))
        b6_s=ld('b6',[INTER,6],b6.ap()); gf_s=ld('gf',[1,4*INTER],gf.ap())
        sc_s=ld('sc',[1,8],sc.ap()); mgbr_s=ld('mgbr',[1,INTER],mgbr.ap())
        osel_s=ld('osel',[NCORES,1],osel.ap())
        ones_c=pp.tile([128,1],F32,tag='ones_c'); nc.vector.memset(ones_c[:],1.0)
        zz=pp.tile([128,135],F32,tag='zz'); nc.vector.memset(zz[:],0.0)
        ONESR=gf_s[0:1,3*INTER:4*INTER]
        # conv -> b_s own rows
        outc=pp.tile([9,WIN],F32,tag='outc')
        for ch in range(5):
            pc=ps_mid.tile([128,512],F32,tag='mid')
            for h in range(2):
                nc.tensor.matmul(pc[:9,:468],we_s[:,h,:],gp_s[:,h,ch*468:(ch+1)*468],
                                 start=(h==0),stop=(h==1))
            nc.scalar.activation(outc[:,ch*468:(ch+1)*468],pc[:9,:468],AF.Copy)
        ov=outc[:].rearrange("p (h w) -> p h w",w=PW)
        bsa=pp.tile([HR,128],F32,tag='bsa')
        for m in range(9):
            kh,kw=divmod(m,3)
            bt=ww.tile([HR,128],F32,tag='bt')
            nc.sync.dma_start(bt[:],ov[m:m+1,kh:kh+HR,kw:kw+128])
            if m==0: nc.vector.tensor_copy(bsa[:],bt[:])
            else: nc.vector.tensor_tensor(bsa[:],bsa[:],bt[:],AL.add)
        dma(ag_in[NB*(INTER+1):AGS],bsa[:])
        # psum6
        p6=ps_six.tile([6,512],F32,tag='six')
        for kt in range(KT):
            nc.tensor.matmul(p6[:,:],xv_s[:,kt,:],xT_s[:,kt,:],start=(kt==0),
                             stop=(kt==KT-1))
        p6sb=pp.tile([6,512],F32,tag='p6sb')
        nc.scalar.activation(p6sb[:],p6[:,:],AF.Copy)
        p6r=[]
        for r in range(6):
            t=pp.tile([1,512],F32,tag=f'p6r{r}',name=f'p6r{r}')
            dma(t[:],p6sb[r:r+1,:]); p6r.append(t)
        s_sbs=[]
        for j in range(3):
            s_sbs.append(pp.tile([1,512],F32,tag=f's_sb{j}',name=f's_sb{j}'))
            psv=ps_mid.tile([128,512],F32,tag='mid')
            for kt in range(KT):
                nc.tensor.matmul(psv[:1,:],vp_s[:,kt,j:j+1],yT_s[:,j,kt,:],
                                 start=(kt==0),stop=(kt==KT-1))
            spre=ww.tile([1,512],F32,tag='spre',bufs=1)
            nc.vector.tensor_scalar(spre[:],psv[:1,:],sc_s[0:1,1+j:2+j],None,AL.add)
            t2=ww.tile([1,512],F32,tag='t2',bufs=1)
            nc.vector.tensor_tensor(t2[:],p6r[1+j][:],spre[:],AL.add)
            nc.scalar.activation(s_sbs[j][:],t2[:],AF.Relu)
        b_sb=pp.tile([1,512],F32,tag='b_sb')
        nc.vector.tensor_scalar(b_sb[:],p6r[5][:],sc_s[0:1,5:6],None,AL.add)
        dma(ag_in[NB*INTER:NB*(INTER+1)],b_sb[:])
        a_sb=pp.tile([1,512],F32,tag='a_sb')
        nc.vector.tensor_scalar(a_sb[:],p6r[0][:],sc_s[0:1,0:1],None,AL.add)
        # g_x row-major
        gxo=pp.tile([128,4,INTER],F32,tag='gxo')
        for i4 in range(4):
            pg=ps_mid.tile([128,512],F32,tag='mid')
            for kt in range(KT):
                nc.tensor.matmul(pg[:,:INTER],xT_s[:,kt,i4*128:(i4+1)*128],mgw_s[:,kt,:],
                                 start=(kt==0),stop=False,skip_group_check=True)
            nc.tensor.matmul(pg[:,:INTER],ONESR,mgbr_s[:],start=False,stop=True,
                             skip_group_check=True)
            nc.scalar.activation(gxo[:,i4,:],pg[:,:INTER],AF.Copy)
        dma(mkap(ag_in[:],0,[(128,128),(16384,4),(1,128)]),gxo[:])
        nc.gpsimd.collective_compute("AllGather",AL.bypass,ins=[ag_in[:].opt()],
                                     outs=[ag_out[:].opt()],replica_groups=RG)
        # softmax + p windows
        bs_f=pp.tile([128,128],F32,tag='bs_f')
        for c in range(NCORES):
            dma(bs_f[c*HR:(c+1)*HR,:],ag_out[c*AGS+NB*(INTER+1):c*AGS+AGS])
        e_sb=pp.tile([128,128],F32,tag='e_sb'); zc=pp.tile([128,1],F32,tag='zc')
        nc.scalar.activation(e_sb[:],bs_f[:],AF.Exp,accum_out=zc[:])
        pz=ps_sm.tile([128,512],F32,tag='sm')
        nc.tensor.matmul(pz[:1,:1],zc[:],ones_c[:],start=True,stop=True)
        z_sb=pp.tile([1,1],F32,tag='z_sb'); nc.vector.tensor_copy(z_sb[:],pz[:1,:1])
        zr=pp.tile([1,1],F32,tag='zr'); nc.vector.reciprocal(zr[:],z_sb[:])
        pzb=ps_sm.tile([128,512],F32,tag='sm')
        nc.tensor.matmul(pzb[:,:1],ONESR,zr[:],start=True,stop=True)
        zrb=pp.tile([128,1],F32,tag='zrb'); nc.vector.tensor_copy(zrb[:],pzb[:,:1])
        dma(p_glob[:],zz[:])
        dma(mkap(p_glob[:],262,[(130,128),(1,128)]),e_sb[:])
        p8=pp.tile([NCORES,2048],F32,tag='p8')
        dma(p8[:],mkap(p_glob[:],262,[(HR*PW,NCORES),(PW,HR),(1,128)]))
        ow_sb=pp.tile([1,2048],F32,tag='ow_sb')
        for ch in range(4):
            pwc=ps_sm.tile([128,512],F32,tag='sm')
            nc.tensor.matmul(pwc[:1,:512],osel_s[:],p8[:,ch*512:(ch+1)*512],
                             start=True,stop=True)
            nc.scalar.activation(ow_sb[:,ch*512:(ch+1)*512],pwc[:1,:512],AF.Copy)
        dma(p_loc[:],zz[:,:22])
        dma(mkap(p_loc[:],262,[(PW,HR),(1,128)]),ow_sb[:])
        # q matmuls
        pq=ps_mid.tile([128,512],F32,tag='mid')
        for t in range(QT):
            lq=ww.tile([128,9],F32,tag='lq')
            dma(lq[:],mkap(p_loc[:],128*t,[(1,128),(130,3),(1,3)]))
            gptt=ww.tile([128,GC],F32,tag='gptt',name=f'gptt{t}',bufs=2)
            dma(gptt[:],gpadT.ap()[t])
            nc.tensor.matmul(pq[:9,:GC],lq[:],gptt[:],start=(t==0),stop=(t==QT-1))
        q_sb=pp.tile([9,GC],F32,tag='q_sb')
        nc.scalar.activation(q_sb[:],pq[:9,:GC],AF.Copy)
        dma(ar_in[0:9*GC],q_sb[:])
        # colsums
        cs_sb=pp.tile([128,KT],F32,tag='cs_sb')
        bafv=bafT.ap().rearrange("(k p) n -> p k n",p=128)
        for kt in range(KT):
            bft=ww.tile([128,2*NB],F32,tag='bft',name=f'bft{kt}',bufs=2)
            dma(bft[:],bafv[:,kt,:])
            nc.vector.tensor_reduce(cs_sb[:,kt:kt+1],bft[:],
                                    axis=mybir.AxisListType.X,op=AL.add)
        dma(ar_in[9*GC:ARS],cs_sb[:].rearrange("p k -> k p"))
        nc.gpsimd.collective_compute("AllReduce",AL.add,ins=[ar_in[:].opt()],
                                     outs=[ar_out[:].opt()],replica_groups=RG)
        # post-AR small matvecs
        pgm=ps_sm.tile([128,512],F32,tag='sm2')
        ppm=ps_sm.tile([128,512],F32,tag='sm')
        for kt in range(KT):
            cst=ww.tile([128,1],F32,tag='cst')
            dma(cst[:],ar_out[9*GC+128*kt:9*GC+128*(kt+1)])
            nc.tensor.matmul(pgm[:,:1],bag_s[:,kt,:],cst[:],start=(kt==0),
                             stop=(kt==KT-1),skip_group_check=True)
            nc.tensor.matmul(ppm[:1,:1],vpm_s[:,kt,:],cst[:],start=(kt==0),
                             stop=(kt==KT-1),skip_group_check=True)
        gm_sb=pp.tile([128,1],F32,tag='gm_sb')
        nc.vector.tensor_scalar(gm_sb[:],pgm[:,:1],b6_s[:,4:5],None,AL.add)
        pm_sb=pp.tile([1,1],F32,tag='pm_sb')
        nc.vector.tensor_scalar(pm_sb[:],ppm[:1,:1],sc_s[0:1,4:5],None,AL.add)
        pv=ps_sm.tile([128,512],F32,tag='sm2')
        for t in range(18):
            qrt=ww.tile([128,1],F32,tag='qrt')
            dma(qrt[:],ar_out[128*t:128*(t+1)])
            nc.tensor.matmul(pv[:,:1],spg_s[:,t,:],qrt[:],start=(t==0),stop=(t==17))
        v_sb=pp.tile([128,1],F32,tag='v_sb')
        nc.vector.tensor_scalar(v_sb[:],pv[:,:1],zrb[:],b6_s[:,5:6],AL.mult,AL.add)
        # s_ba + broadcasts + pair terms
        sba=pp.tile([1,512],F32,tag='sba')
        nc.scalar.activation(sba[:],p6r[4][:],AF.Relu,bias=pm_sb[0:1,0:1])
        acc=pp.tile([128,512],F32,tag='acc')
        tmp=pp.tile([128,512],F32,tag='tmp')
        for j in range(3):
            py=ps_roll.tile([128,512],F32,tag='roll')
            for kt in range(KT):
                nc.tensor.matmul(py[:,:],pr_s[:,j,kt,:],yT_s[:,j,kt,:],
                                 start=(kt==0),stop=(kt==KT-1))
            gy=ww.tile([128,512],F32,tag='gy',bufs=1)
            nc.vector.tensor_scalar(gy[:],py[:,:],b6_s[:,1+j:2+j],None,AL.add)
            pb=ps_roll.tile([128,512],F32,tag='roll')
            nc.tensor.matmul(pb[:,:],gf_s[0:1,j*INTER:(j+1)*INTER],s_sbs[j][:],
                             start=True,stop=True)
            if j==0:
                nc.vector.tensor_tensor(acc[:],gy[:],pb[:,:],AL.mult)
            else:
                nc.vector.tensor_tensor(tmp[:],gy[:],pb[:,:],AL.mult)
                nc.vector.tensor_tensor(acc[:],acc[:],tmp[:],AL.add)
        psb=ps_roll.tile([128,512],F32,tag='roll')
        nc.tensor.matmul(psb[:,:],ONESR,sba[:],start=True,stop=True)
        nc.vector.tensor_scalar(tmp[:],psb[:,:],gm_sb[:],None,AL.mult)
        nc.vector.tensor_tensor(acc[:],acc[:],tmp[:],AL.add)
        pab=ps_roll.tile([128,512],F32,tag='roll')
        nc.tensor.matmul(pab[:,:],ONESR,a_sb[:],start=True,stop=True)
        ab_sb=pp.tile([128,512],F32,tag='ab_sb')
        nc.scalar.activation(ab_sb[:],pab[:,:],AF.Copy)
        # gx readback + origin loop
        gx_sb=pp.tile([128,32,128],F32,tag='big',name='gx_sb')
        for c in range(NCORES):
            dma(gx_sb[:,4*c:4*(c+1),:],mkap(ag_out[:],c*AGS,[(128,128),(16384,4),(1,128)]))
        po=ps_or.tile([128,512],F32,tag='orig')
        for jt in range(32):
            cc,lt=divmod(jt,4)
            bc=ww.tile([128,1],F32,tag='bc')
            dma(bc[:],ag_out[cc*AGS+NB*INTER+lt*128:cc*AGS+NB*INTER+(lt+1)*128])
            fT=ww.tile([128,512],F32,tag='fT',bufs=2)
            if jt%8<3:
                nc.scalar.activation(fT[:],ab_sb[:],AF.Relu,bias=bc[:])
            else:
                nc.vector.tensor_scalar(fT[:],ab_sb[:],bc[:],0.0,AL.add,AL.max)
            nc.tensor.matmul(po[:,:],gx_sb[:,jt,:],fT[:],start=(jt==0),stop=(jt==31))
        ot=pp.tile([128,512],F32,tag='ot')
        nc.vector.tensor_scalar(ot[:],po[:,:],1.0/N,v_sb[:],AL.mult,AL.add)
        fin=pp.tile([128,512],F32,tag='fin')
        nc.vector.tensor_tensor(fin[:],acc[:],ot[:],AL.add)
        dma(out_ext.ap(),fin[:])
    nc.compile()
    return nc



# revision 10
# speedup vs baseline: 2.0425x; 2.0425x over previous
import numpy as np
import ml_dtypes

N=4096; C=1024; INTER=128; R=128; GC=256; NCORES=8; NB=N//NCORES
HR=R//NCORES; PW=130; WINR=HR+2; WIN=WINR*PW; QT=19; KT=C//128
GXE=NB*INTER; BOFF=GXE; QOFF=GXE+NB; ZOFF=QOFF+2304; AGE=QOFF+2432  # 68480
BF=ml_dtypes.bfloat16

_cache = {}

def _fold(p):
    f32=np.float32
    out={}
    mcw1=np.asarray(p['m_cw'][:INTER],f32); mcw2=np.asarray(p['m_cw'][INTER:],f32)
    xv=np.zeros((C,6),f32); sc=np.zeros((1,8),f32)
    xv[:,0]=np.asarray(p['m_tw'],f32).T@mcw1; sc[0,0]=np.asarray(p['m_tb'],f32)@mcw1
    for j in range(3):
        c1=np.asarray(p['pr_cw'][j,:INTER],f32); c2=np.asarray(p['pr_cw'][j,INTER:],f32)
        xv[:,1+j]=np.asarray(p['pr_tw'][j],f32).T@c1
        sc[0,1+j]=np.asarray(p['pr_tb'][j],f32)@c1+np.asarray(p['pr_pb'][j],f32)@c2
    bc1=np.asarray(p['ba_cw'][:INTER],f32); bc2=np.asarray(p['ba_cw'][INTER:],f32)
    xv[:,4]=np.asarray(p['ba_tw'],f32).T@bc1
    xv[:,5]=np.asarray(p['m_pw'],f32).T@mcw2; sc[0,5]=np.asarray(p['m_pb'],f32)@mcw2
    # bef/aft enter only via column means: fold on host
    cs=(np.asarray(p['bef'],f32).sum(0)+np.asarray(p['aft'],f32).sum(0))/(2.0*N)
    sc[0,4]=(np.asarray(p['ba_tb'],f32)@bc1+np.asarray(p['ba_pb'],f32)@bc2
             +cs@(np.asarray(p['ba_pw'],f32).T@bc2))
    out['sc']=sc
    out['xv']=np.ascontiguousarray(xv.reshape(KT,128,6).transpose(1,0,2)).astype(BF)
    vps=np.stack([np.asarray(p['pr_pw'][j],f32).T@np.asarray(p['pr_cw'][j,INTER:],f32)
                  for j in range(3)],1)
    out['vp']=np.ascontiguousarray(vps.reshape(KT,128,3).transpose(1,0,2)).astype(BF)
    out['mgw']=np.ascontiguousarray(
        np.asarray(p['m_gw'],f32).T.reshape(KT,128,INTER).transpose(1,0,2)).astype(BF)
    prw=np.stack([np.asarray(p['pr_gw'][j],f32).T for j in range(3)])
    out['prw']=np.ascontiguousarray(
        prw.reshape(3,KT,128,INTER).transpose(2,0,1,3)).astype(BF)
    bg=float(np.asarray(p['ba_g'],f32)[0]); sg=float(np.asarray(p['sp_g'],f32)[0])
    gm=bg*(cs@np.asarray(p['ba_gw'],f32).T+np.asarray(p['ba_gb'],f32))
    b6=np.zeros((INTER,6),f32)
    b6[:,1:4]=np.asarray(p['pr_gb'],f32).T; b6[:,4]=gm
    b6[:,5]=sg*np.asarray(p['sp_gb'],f32)
    out['b6']=b6
    g=np.transpose(np.asarray(p['sp_gw'],f32),(2,3,1,0))[::-1,::-1]
    spg=sg*np.ascontiguousarray(g).reshape(9*GC,INTER)
    out['spg']=np.ascontiguousarray(spg.reshape(18,128,INTER).transpose(1,0,2)).astype(BF)
    we=np.einsum('c,cikl->ikl',np.asarray(p['sp_cw'][INTER:],f32),np.asarray(p['sp_pw'],f32))
    out['wef']=np.ascontiguousarray(we.reshape(2,128,9).transpose(1,0,2)).astype(BF)
    gf=np.zeros((1,4*INTER),f32)
    for j in range(3): gf[0,j*INTER:(j+1)*INTER]=np.asarray(p['pr_g'],f32)[j]
    gf[0,3*INTER:]=1.0
    out['gf']=gf.astype(BF)
    out['mgbr']=np.asarray(p['m_gb'],f32)[None,:].astype(BF)
    return out

def _shard(p):
    f32=np.float32
    gpadded=np.pad(np.asarray(p['global_feature'],f32)[0],((0,0),(1,1),(1,1)))
    x=np.asarray(p['origin_feature'],f32)
    ys=[np.asarray(p[t],f32) for t in ('local_feature','bef_l','aft_l')]
    ins=[]
    for k in range(NCORES):
        d={}
        rs=slice(k*NB,(k+1)*NB)
        d['xT']=np.ascontiguousarray(
            x[rs].T.reshape(KT,128,NB).transpose(1,0,2)).astype(BF)
        yb=np.stack([np.ascontiguousarray(y[rs].T).reshape(KT,128,NB) for y in ys])
        d['yT']=np.ascontiguousarray(yb.transpose(2,0,1,3)).astype(BF)
        gw=gpadded[:,k*HR:k*HR+WINR,:]                      # [GC,18,130]
        d['gps']=np.ascontiguousarray(
            gw.reshape(2,128,WIN).transpose(1,0,2)).astype(BF)
        gt=np.zeros((QT*128,GC),f32); gt[:WIN]=gw.reshape(GC,WIN).T
        d['gpt']=np.ascontiguousarray(
            gt.reshape(QT,128,GC).transpose(1,0,2)).astype(BF)
        ins.append(d)
    return ins

def kernel(**inputs):
    if 'nc' not in _cache:
        _cache['nc']=build()
    nc=_cache['nc']
    fold=_fold(inputs); shards=_shard(inputs)
    in_maps=[]
    for k in range(NCORES):
        m=dict(shards[k]); m.update(fold)
        in_maps.append({kk:np.ascontiguousarray(v) for kk,v in m.items()})
    from concourse.bass_utils import run_bass_kernel_spmd
    res=run_bass_kernel_spmd(nc,in_maps,list(range(NCORES)))
    out=np.empty((N,INTER),np.float32)
    for k in range(NCORES):
        out[k*NB:(k+1)*NB]=res.results[k]['out'].T
    return out


# ---- device program builder ----
import bass_rust
import concourse.bass as bass
import concourse.bacc as bacc
import concourse.mybir as mybir
import concourse.tile as tile

F32=mybir.dt.float32
BF16=mybir.dt.bfloat16
AF=mybir.ActivationFunctionType
AL=mybir.AluOpType
RG=[list(range(NCORES))]

def mkap(a,offset,dims):
    b=a.copy(); b.offset=offset
    b.ap=bass_rust.VecI64Pair([list(d) for d in dims])
    return b

def build():
    nc=bacc.Bacc("TRN2",target_bir_lowering=False,debug=False,num_devices=NCORES)
    def P(n,s,dt=BF16): return nc.declare_dram_parameter(n,list(s),dt,isOutput=False)
    xT=P('xT',(128,KT,NB)); yT=P('yT',(128,3,KT,NB))
    gps=P('gps',(128,2,WIN)); gpt=P('gpt',(128,QT,GC))
    xv=P('xv',(128,KT,6)); vp=P('vp',(128,KT,3)); mgw=P('mgw',(128,KT,INTER))
    prw=P('prw',(128,3,KT,INTER)); spg=P('spg',(128,18,INTER)); wef=P('wef',(128,2,9))
    b6=P('b6',(INTER,6),F32); gf=P('gf',(1,512)); sc=P('sc',(1,8),F32)
    mgbr=P('mgbr',(1,INTER))
    out_ext=nc.declare_dram_parameter('out',[INTER,NB],F32,isOutput=True)

    with tile.TileContext(nc) as tc:
      with (tc.tile_pool(name="pp",bufs=1) as pp,
            tc.tile_pool(name="ww",bufs=4) as ww,
            tc.tile_pool(name="dr",bufs=1,space="DRAM") as dr,
            tc.tile_pool(name="ps_or",bufs=1,space="PSUM") as ps_or,
            tc.tile_pool(name="ps_six",bufs=1,space="PSUM") as ps_six,
            tc.tile_pool(name="ps_mid",bufs=2,space="PSUM") as ps_mid,
            tc.tile_pool(name="ps_roll",bufs=3,space="PSUM") as ps_roll):
        ag_in=dr.tile([AGE],BF16); ag_out=dr.tile([NCORES*AGE],BF16,addr_space='Shared')
        ploc=dr.tile([2944],BF16)
        sdma=nc.sync.dma_start; cdma=nc.scalar.dma_start; vdma=nc.gpsimd.dma_start
        def ld(q,name,shape,src_ap,dt=BF16):
            t=pp.tile(shape,dt,tag=name,name=name)
            q(t[:],src_ap)
            return t
        # critical-path queue (sync)
        xv_s=ld(sdma,'xv',[128,KT,6],xv.ap())
        mgw_s=ld(sdma,'mgw',[128,KT,INTER],mgw.ap())
        xT_s=ld(sdma,'xT',[128,KT,NB],xT.ap())
        mgbr_s=ld(sdma,'mgbr',[1,INTER],mgbr.ap())
        gf_s=ld(sdma,'gf',[1,512],gf.ap())
        sc_s=ld(sdma,'sc',[1,8],sc.ap(),F32)
        b6_s=ld(sdma,'b6',[INTER,6],b6.ap(),F32)
        # conv/spatial queue (scalar)
        wef_s=ld(cdma,'wef',[128,2,9],wef.ap())
        gps_s=ld(cdma,'gps',[128,2,WIN],gps.ap())
        gpt_s=ld(cdma,'gpt',[128,QT,GC],gpt.ap())
        spg_s=ld(cdma,'spg',[128,18,INTER],spg.ap())
        # pair queue (vector)
        vp_s=ld(vdma,'vp',[128,KT,3],vp.ap())
        yT_s=pp.tile([128,3,KT,NB],BF16,tag='yT')
        for j in range(3):
            vdma(yT_s[:,j,:,:],yT.ap()[:,j])
        pr_s=ld(vdma,'pr',[128,3,KT,INTER],prw.ap())
        ONESR=gf_s[0:1,3*INTER:4*INTER]
        ONE1=gf_s[0:1,3*INTER:3*INTER+1]
        zz=pp.tile([1,608],BF16,tag='zz'); nc.vector.memset(zz[:],0.0)
        ones16=pp.tile([16,1],F32,tag='ones16'); nc.vector.memset(ones16[:],1.0)
        onesf=pp.tile([1,INTER],F32,tag='onesf'); nc.vector.memset(onesf[:],1.0)
        e_sb=pp.tile([16,PW],BF16,tag='e_sb'); nc.vector.memset(e_sb[:],0.0)
        # ---- psum6: 6 folded x-dot-products ----
        p6=ps_six.tile([6,512],F32,tag='six')
        for kt in range(KT):
            nc.tensor.matmul(p6[:,:],xv_s[:,kt,:],xT_s[:,kt,:],start=(kt==0),
                             stop=(kt==KT-1))
        p6sb=pp.tile([6,512],F32,tag='p6sb')
        nc.scalar.activation(p6sb[:],p6[:,:],AF.Copy)
        p6r=[]
        for r in range(6):
            t=pp.tile([1,512],F32,tag=f'p6r{r}',name=f'p6r{r}')
            sdma(t[:],p6sb[r:r+1,:]); p6r.append(t)
        a_b=pp.tile([1,512],BF16,tag='a_b')
        nc.vector.tensor_scalar(a_b[:],p6r[0][:],sc_s[0:1,0:1],None,AL.add)
        b_b=pp.tile([1,512],BF16,tag='b_b')
        nc.vector.tensor_scalar(b_b[:],p6r[5][:],sc_s[0:1,5:6],None,AL.add)
        sdma(ag_in[BOFF:BOFF+NB],b_b[:])
        sba=pp.tile([1,512],BF16,tag='sba')
        nc.scalar.activation(sba[:],p6r[4][:],AF.Relu,bias=sc_s[0:1,4:5])
        # ---- g_x (own rows, row-major) ----
        gxo=pp.tile([128,4,INTER],BF16,tag='gxo')
        for i4 in range(4):
            pg=ps_mid.tile([128,512],F32,tag='mid')
            for kt in range(KT):
                nc.tensor.matmul(pg[:,:INTER],xT_s[:,kt,i4*128:(i4+1)*128],mgw_s[:,kt,:],
                                 start=(kt==0),stop=False,skip_group_check=True)
            nc.tensor.matmul(pg[:,:INTER],ONESR,mgbr_s[:],start=False,stop=True,
                             skip_group_check=True)
            nc.scalar.activation(gxo[:,i4,:],pg[:,:INTER],AF.Copy)
        sdma(mkap(ag_in[:],0,[(128,128),(16384,4),(1,128)]),gxo[:])
        # ---- conv -> b_s rows (own spatial window) ----
        outc=pp.tile([9,WIN],BF16,tag='outc')
        for ch in range(5):
            pc=ps_mid.tile([128,512],F32,tag='mid')
            for h in range(2):
                nc.tensor.matmul(pc[:9,:468],wef_s[:,h,:],gps_s[:,h,ch*468:(ch+1)*468],
                                 start=(h==0),stop=(h==1))
            nc.scalar.activation(outc[:,ch*468:(ch+1)*468],pc[:9,:468],AF.Copy)
        ov=outc[:].rearrange("p (h w) -> p h w",w=PW)
        bsa=pp.tile([HR,128],F32,tag='bsa')
        for m in range(9):
            kh,kw=divmod(m,3)
            bt=ww.tile([HR,128],BF16,tag='bt')
            sdma(bt[:],ov[m:m+1,kh:kh+HR,kw:kw+128])
            if m==0: nc.vector.tensor_copy(bsa[:],bt[:])
            else: nc.vector.tensor_tensor(bsa[:],bsa[:],bt[:],AL.add)
        zc=pp.tile([16,1],F32,tag='zc')
        nc.scalar.activation(e_sb[:,0:128],bsa[:],AF.Exp,accum_out=zc[:])
        pz=ps_roll.tile([128,512],F32,tag='roll')
        nc.tensor.matmul(pz[:1,:1],zc[:],ones16[:],start=True,stop=True)
        z_b=pp.tile([1,1],BF16,tag='z_b'); nc.vector.tensor_copy(z_b[:],pz[:1,:1])
        sdma(ag_in[ZOFF:ZOFF+1],z_b[:])
        # unnormalized p window -> ploc (guards zeroed), windowed lq gather
        cdma(ploc[0:262],zz[0:1,0:262])
        cdma(ploc[2342:2944],zz[0:1,0:602])
        cdma(mkap(ploc[:],262,[(130,16),(1,130)]),e_sb[:])
        lq_s=pp.tile([128,QT,9],BF16,tag='lq_s')
        for dr in range(3):
            cdma(lq_s[:,:,3*dr:3*dr+3],mkap(ploc[:],130*dr,[(1,128),(128,19),(1,3)]))
        # ---- q correlation ----
        pq=ps_mid.tile([128,512],F32,tag='mid')
        for t in range(QT):
            nc.tensor.matmul(pq[:9,:GC],lq_s[:,t,:],gpt_s[:,t,:],start=(t==0),
                             stop=(t==QT-1))
        q_sb=pp.tile([9,GC],BF16,tag='q_sb')
        nc.scalar.activation(q_sb[:],pq[:9,:GC],AF.Copy)
        sdma(ag_in[QOFF:QOFF+2304],q_sb[:])
        # ---- a broadcast ----
        pab=ps_roll.tile([128,512],F32,tag='roll')
        nc.tensor.matmul(pab[:,:],ONESR,a_b[:],start=True,stop=True)
        ab_sb=pp.tile([128,512],BF16,tag='ab_sb')
        nc.scalar.activation(ab_sb[:],pab[:,:],AF.Copy)
        # ---- single collective ----
        nc.gpsimd.collective_compute("AllGather",AL.bypass,ins=[ag_in[:].opt()],
                                     outs=[ag_out[:].opt()],replica_groups=RG)
        # ---- during-collective: pair units + ba ----
        s_bs=[]
        for j in range(3):
            psv=ps_roll.tile([128,512],F32,tag='roll')
            for kt in range(KT):
                nc.tensor.matmul(psv[:1,:],vp_s[:,kt,j:j+1],yT_s[:,j,kt,:],
                                 start=(kt==0),stop=(kt==KT-1))
            spre=ww.tile([1,512],F32,tag='spre',bufs=2)
            nc.vector.tensor_scalar(spre[:],psv[:1,:],sc_s[0:1,1+j:2+j],None,AL.add)
            t2=ww.tile([1,512],F32,tag='t2',bufs=2)
            nc.vector.tensor_tensor(t2[:],p6r[1+j][:],spre[:],AL.add)
            sb=pp.tile([1,512],BF16,tag=f's_b{j}',name=f's_b{j}')
            nc.scalar.activation(sb[:],t2[:],AF.Relu); s_bs.append(sb)
        acc=pp.tile([128,512],F32,tag='acc')
        tmp=pp.tile([128,512],F32,tag='tmp')
        for j in range(3):
            py=ps_roll.tile([128,512],F32,tag='roll')
            for kt in range(KT):
                nc.tensor.matmul(py[:,:],pr_s[:,j,kt,:],yT_s[:,j,kt,:],
                                 start=(kt==0),stop=(kt==KT-1))
            gy=ww.tile([128,512],F32,tag='gy',bufs=2)
            nc.vector.tensor_scalar(gy[:],py[:,:],b6_s[:,1+j:2+j],None,AL.add)
            pb=ps_roll.tile([128,512],F32,tag='roll')
            nc.tensor.matmul(pb[:,:],gf_s[0:1,j*INTER:(j+1)*INTER],s_bs[j][:],
                             start=True,stop=True)
            if j==0:
                nc.vector.tensor_tensor(acc[:],gy[:],pb[:,:],AL.mult)
            else:
                nc.vector.tensor_tensor(tmp[:],gy[:],pb[:,:],AL.mult)
                nc.vector.tensor_tensor(acc[:],acc[:],tmp[:],AL.add)
        psb=ps_roll.tile([128,512],F32,tag='roll')
        nc.tensor.matmul(psb[:,:],ONESR,sba[:],start=True,stop=True)
        nc.vector.tensor_scalar(tmp[:],psb[:,:],b6_s[:,4:5],None,AL.mult)
        nc.vector.tensor_tensor(acc[:],acc[:],tmp[:],AL.add)
        # ---- post-collective readback ----
        bc_b=pp.tile([128,32],BF16,tag='bc_b')
        for c in range(NCORES):
            sdma(bc_b[:,4*c:4*c+4],mkap(ag_out[:],c*AGE+BOFF,[(1,128),(128,4)]))
        bc_s=pp.tile([128,32],F32,tag='bc_s')
        nc.vector.tensor_copy(bc_s[:],bc_b[:])
        gx_sb=pp.tile([128,32,128],BF16,tag='gx_sb')
        for c in range(NCORES):
            q=sdma if c%2==0 else cdma
            q(gx_sb[:,4*c:4*(c+1),:],mkap(ag_out[:],c*AGE,[(128,128),(16384,4),(1,128)]))
        q8=pp.tile([128,8,QT],BF16,tag='q8')
        for c in range(NCORES):
            vdma(q8[:,c,:],mkap(ag_out[:],c*AGE+QOFF,[(1,128),(128,19)]))
        s1=pp.tile([128,4,QT],BF16,tag='s1')
        nc.vector.tensor_tensor(s1[:],q8[:,0:4,:],q8[:,4:8,:],AL.add)
        s2=pp.tile([128,2,QT],BF16,tag='s2')
        nc.vector.tensor_tensor(s2[:],s1[:,0:2,:],s1[:,2:4,:],AL.add)
        qr=pp.tile([128,QT],BF16,tag='qr')
        nc.vector.tensor_tensor(qr[:],s2[:,0,:],s2[:,1,:],AL.add)
        zr=pp.tile([1,1],F32,tag='zr')
        nc.vector.reciprocal(zr[:],qr[0:1,18:19])
        pzb=ps_roll.tile([128,512],F32,tag='roll')
        nc.tensor.matmul(pzb[:,:1],onesf[:],zr[:],start=True,stop=True)
        zrb=pp.tile([128,1],F32,tag='zrb'); nc.vector.tensor_copy(zrb[:],pzb[:,:1])
        pvr=ps_roll.tile([128,512],F32,tag='roll')
        for t in range(18):
            nc.tensor.matmul(pvr[:1,:INTER],qr[:,t:t+1],spg_s[:,t,:],
                             start=(t==0),stop=(t==17))
        v_row=pp.tile([1,INTER],BF16,tag='v_row')
        nc.scalar.activation(v_row[:],pvr[:1,:INTER],AF.Copy)
        pvT=ps_roll.tile([128,512],F32,tag='roll')
        nc.tensor.matmul(pvT[:,:1],v_row[:],ONE1,start=True,stop=True)
        v_sb=pp.tile([128,1],F32,tag='v_sb')
        nc.vector.tensor_scalar(v_sb[:],pvT[:,:1],zrb[:],b6_s[:,5:6],AL.mult,AL.add)
        # ---- main origin loop ----
        po=ps_or.tile([128,512],F32,tag='orig')
        for jt in range(32):
            fT=ww.tile([128,512],BF16,tag='fT',bufs=3)
            if jt%8<3:
                nc.scalar.activation(fT[:],ab_sb[:],AF.Relu,bias=bc_s[:,jt:jt+1])
            else:
                nc.vector.tensor_scalar(fT[:],ab_sb[:],bc_s[:,jt:jt+1],0.0,AL.add,AL.max)
            nc.tensor.matmul(po[:,:],gx_sb[:,jt,:],fT[:],start=(jt==0),stop=(jt==31),
                             skip_group_check=True)
        ot=pp.tile([128,512],F32,tag='ot')
        nc.vector.tensor_scalar(ot[:],po[:,:],1.0/N,v_sb[:],AL.mult,AL.add)
        fin=pp.tile([128,512],F32,tag='fin')
        nc.vector.tensor_tensor(fin[:],acc[:],ot[:],AL.add)
        sdma(out_ext.ap(),fin[:])
    nc.compile()
    return nc
